# revision 42
# baseline (speedup 1.0000x reference)
"""Trainium2 Bass kernel for BatchedDifferentiableDynamicBicycleModel.

Contract: kernel(state=[B,9] f32, action=[B,2] f32, dt=scalar) -> [B,9] f32.
B = 262144, batch-parallel across 8 NeuronCores (32768 vehicles/core, one
[128, 256] tile per state variable, FD=256).

dt=1 -> 100 fp32 Euler substeps (all h = f32(0.01)). v2 design:

  PSUM accumulators (exact fp32, fed by fp16 matmuls):
    [x|y] += h*[v*cos|v*sin],  psi += h*r,  rhat += k1*Tf + k2*Tr,
    [X|Y] = per-step tanh args (scratch),   S1 = c1*Tf + c2*Tr (scratch).
  SBUF fp16 ping-pong states: beta (beta' = beta + w - h*r via two 2x TTs),
    phi (phi' = phi + w; PHISTEP wrap every 8th step only -- the ACT sin
    table is accurate to |x| <= 3.3, and |phi| drifts <= 0.016/step).
  SBUF fp32 ping-pong: v (RELUADD custom), hA = h*a (gpsimd STT decay).
  delta: closed form d_k = dref + q^{k-1}*(d1 - dref) (clip binds only at
    step 1); enters X via per-step decayed diag weights on a constant
    operand. No per-step delta op at all.
  reciprocal: seedless reciprocal_approx_fast (bit-trick + 2 inline NR,
    ~3e-6 rel err, one DVE op). rv16 = (h*r)*inv via the hr16 mirror.
  hr16 = identity-activation(scale=h) of rhat PSUM -> one ACT op feeding
    psi-mm, beta-sub, and rv.
  ACT: sin, cos(=sin(pi/2-|phi|)), one contiguous 2FD tanh, hr16.
  GPSIMD: vcvs = v*[cos|sin] (leaf), hA decay STT (off critical path).
"""

import math
import os
import sys

for _p in ("/opt/trn_rl_repo", "/opt/pypackages"):
    if _p not in sys.path:
        sys.path.insert(0, _p)

import numpy as np

# ----------------------------------------------------------------------------
# Model constants (match reference.py in float64)
# ----------------------------------------------------------------------------
M_, IZ, LF, LR, CF, CR = 1500.0, 2250.0, 1.2, 1.6, 80000.0, 80000.0
TAU_A, TAU_D = 0.1, 0.1
MAX_STEER = 30.0 * np.pi / 180.0
MAX_ACC, MIN_ACC = 3.0, -6.0
MU, G = 0.9, 9.81
L = LF + LR
FY_F_MAX = MU * M_ * G * (LR / L)
FY_R_MAX = MU * M_ * G * (LF / L)
DT_INTERNAL = 0.01
V_EFF_MIN = 20.0 / 3.6

N_CORES = 8
B_TOTAL = 262144
B_CORE = B_TOTAL // N_CORES  # 32768
P = 128
WRAP_EVERY = 8

_f32 = np.float32

# ----------------------------------------------------------------------------
# Custom DVE ops (reused from v1)
# ----------------------------------------------------------------------------
_REG = {}


def _register_custom_ops():
    import concourse.dve_ops as dom
    from concourse.dve_ops import DveOp
    from concourse.dve_spec import (
        Spec, Src0, Src1, C0, C1, C2, lower, maxx, minn, relu, _has_src1,
    )
    from concourse.dve_uop import DveOpSpec

    def reg(name, spec):
        if name in dom._SUB_OPCODE_FOR_NAME:
            _REG[name] = next(op for op in dom.OPS if op.name == name)
            return
        opcode = dom._CUSTOM_DVE_ROW_BASE + len(dom.OPS)
        assert opcode < 0x20, "custom DVE row overflow"
        dom._SUB_OPCODE_FOR_NAME[name] = opcode
        shas = {}
        for ver in ("v3", "v4"):
            s = DveOpSpec(name=name, opcode=opcode, uops=lower(spec, ver=ver),
                          rd1_en=_has_src1(spec))
            shas[ver] = s.sha(ver)
        op = DveOp(name, spec, subdim=False, uops_sha=shas)
        dom.OPS.append(op)
        dom.CUSTOM_DVE_SPECS[name] = spec
        _REG[name] = op

    # phi' = wrap_pm_pi(phi + w): y = in0+in1; y + imm2*((y<-s0)-(y>s0))
    def _phistep_ref(in0, in1, s0, s1, imm2):
        y = (in0.astype(np.float32) + in1).astype(np.float32)
        lo = (y < -s0).astype(np.float32)
        hi = (y > s0).astype(np.float32)
        return (y + imm2 * (lo - hi)).astype(np.float32)

    _y = Src0 + Src1
    reg("ANT_BIKE_PHI_STEP", Spec(body=_y + C2 * ((_y < -C0) - (_y > C0)),
                                  reference=_phistep_ref))

    # full wrap to [-pi,pi]: k = rn(x*s0) via magic s1; out = x - k*imm2
    def _wraprn_ref(in0, in1, s0, s1, imm2):
        t = (in0 * s0).astype(np.float32)
        k = ((t + s1).astype(np.float32) - s1).astype(np.float32)
        return (in0 - k * imm2).astype(np.float32)

    _k = (Src0 * C0 + C1) - C1
    reg("ANT_BIKE_WRAP_RN", Spec(body=Src0 - _k * C2, reference=_wraprn_ref))

    # delta' = clip(delta*s0 + dref*imm2, s1, -s1)  (s1 = -MAX_STEER)
    def _dclip_ref(in0, in1, s0, s1, imm2):
        z = (in0 * s0 + in1 * imm2).astype(np.float32)
        return np.minimum(np.maximum(z, s1), -np.float32(s1)).astype(np.float32)

    _z = Src0 * C0 + Src1 * C2
    reg("ANT_BIKE_DCLIP", Spec(body=minn(maxx(_z, C1), -C1),
                               reference=_dclip_ref))

    # v' = relu(in0 + in1*s0)
    def _reluadd_ref(in0, in1, s0, s1, imm2):
        z = (in0.astype(np.float32) + in1 * s0).astype(np.float32)
        return np.maximum(np.nan_to_num(z, nan=0.0, posinf=np.inf,
                                        neginf=-np.inf), 0).astype(np.float32)

    reg("ANT_BIKE_RELUADD", Spec(body=relu(Src0 + Src1 * C0),
                                 reference=_reluadd_ref))


# ----------------------------------------------------------------------------
# Kernel builder
# ----------------------------------------------------------------------------

def _step_hs(dt_total):
    """Replicate the reference's python-float substep splitting."""
    n_full = int(dt_total // DT_INTERNAL)
    dt_rem = dt_total - n_full * DT_INTERNAL
    hs = [DT_INTERNAL] * n_full
    if dt_rem > 0.0:
        hs.append(dt_rem)
    return hs


def build_kernel(hs, n_veh=B_CORE):
    _register_custom_ops()
    import concourse.bacc as bacc
    import concourse.bass as bass
    import concourse.tile as tile
    from concourse import mybir
    from concourse.mybir import AluOpType as alu
    ACT = mybir.ActivationFunctionType

    FD = n_veh // P
    n_steps = len(hs)
    assert n_steps >= 1

    hs32 = [_f32(h) for h in hs]
    h_base = float(hs32[0])
    MS = _f32(MAX_STEER)
    VMIN = _f32(V_EFF_MIN)
    CFS = float(_f32(-CF / FY_F_MAX))   # tanh arg scale front (negative)
    CRS = float(_f32(-CR / FY_R_MAX))
    PI_F = _f32(np.pi)
    TWO_PI = _f32(2.0 * np.pi)
    INV_2PI = _f32(1.0 / (2.0 * np.pi))
    MAGIC = _f32(12582912.0)
    HALF_PI = _f32(np.pi / 2.0)

    # per-step a/delta decay factors (f32-faithful)
    QA = [float(_f32(1.0) - _f32(float(h) / TAU_A)) for h in hs32]
    QD = [float(_f32(1.0) - _f32(float(h) / TAU_D)) for h in hs32]
    # prefix products: delta_k (k>=1) = dref + (prod_{j=1}^{k-1} QD_j) * D
    ppi = [1.0]
    for k in range(1, n_steps):
        ppi.append(ppi[-1] * QD[k])

    # ---- diag weight set (fp16) ----
    dset = []

    def widx(c):
        c16 = float(np.float16(c))
        dset.append(c16)
        return len(dset) - 1

    D_ONE = widx(1.0)            # [psi|beta] += [h*r | w]
    D_NEG1 = widx(-1.0)          # beta -= h*r
    D_CF = widx(CFS)             # beta -> X
    D_CR = widx(CRS)             # beta -> Y
    D_NCF = widx(-CFS)           # dref/delta0 -> X
    hdi = {}
    for h32 in sorted(set(float(v) for v in hs32)):
        h = float(h32)
        hdi[h] = {
            "XY": widx(h),                            # vcvs -> [x|y]
            "RVF": widx(CFS * LF / h),                # rv16 -> X
            "RVR": widx(-CRS * LR / h),               # rv16 -> Y
            "C1": widx(float(_f32(h * FY_F_MAX / M_))),
            "C2": widx(float(_f32(h * FY_R_MAX / M_))),
            "K1": widx(float(_f32(h * LF * FY_F_MAX / IZ))),
            "K2": widx(float(_f32(-h * LR * FY_R_MAX / IZ))),
        }
    # decayed delta weights for steps k>=1: -CFS * ppi[k-1] ... note the X
    # contribution is (+CFS)*(-delta) => weight on delta operand is -CFS.
    D_DEC = []
    for k in range(1, n_steps):
        D_DEC.append(widx(-CFS * ppi[k - 1]))
    ND = len(dset)

    wdiag_host = np.zeros((ND, P, P), dtype=np.float16)
    eye = np.eye(P, dtype=np.float16)
    for i, c in enumerate(dset):
        wdiag_host[i] = eye * np.float16(c)

    nc = bacc.Bacc("TRN2", target_bir_lowering=False, debug=False)
    st_d = nc.declare_dram_parameter("state", [n_veh, 9], mybir.dt.float32,
                                     isOutput=False)
    ac_d = nc.declare_dram_parameter("action", [n_veh, 2], mybir.dt.float32,
                                     isOutput=False)
    wd_d = nc.declare_dram_parameter("wdiag", [ND, P, P], mybir.dt.float16,
                                     isOutput=False)
    out_d = nc.declare_dram_parameter("out", [n_veh, 9], mybir.dt.float32,
                                      isOutput=True)
    DBG = bool(int(os.environ.get("BIKE_DBG", "0")))
    if DBG:
        dbg_d = nc.declare_dram_parameter("dbg", [n_veh, 8],
                                          mybir.dt.float32, isOutput=True)

    f32 = mybir.dt.float32
    f16 = mybir.dt.float16
    u16 = mybir.dt.uint16

    PHISTEP = _REG["ANT_BIKE_PHI_STEP"]
    WRAPRN = _REG["ANT_BIKE_WRAP_RN"]
    DCLIP = _REG["ANT_BIKE_DCLIP"]
    RELUADD = _REG["ANT_BIKE_RELUADD"]

    with tile.TileContext(nc) as tc:
        with (
            tc.tile_pool(name="persist", bufs=1) as pp,
            tc.tile_pool(name="scratch", bufs=6) as sp,
            tc.tile_pool(name="psum", bufs=1, space="PSUM") as qq,
        ):
            # ---------- persistent SBUF ----------
            big_in = pp.tile([P, FD * 9], f32)
            big_ac = pp.tile([P, FD * 2], f32)
            big_out = pp.tile([P, FD * 9], f32)
            wsb = pp.tile([P, ND * P], f16)
            # v (8 bufs: the lagging gpsimd vcvs reads old bufs; deep
            # rotation keeps its WAR waits off the Vector queue)
            v_t = [pp.tile([P, FD], f32, name=f"v{i}") for i in range(8)]
            hA_t = [pp.tile([P, FD], f32, name=f"hA{i}") for i in range(2)]
            phi_t = [pp.tile([P, FD], f16, name=f"phi{i}") for i in range(3)]
            CA32 = pp.tile([P, FD], f32)       # (h^2/tau_a)*aref
            dref_c = pp.tile([P, FD], f32)
            dref16 = pp.tile([P, FD], f16)
            D32 = pp.tile([P, FD], f32)        # d1 - dref
            D16 = pp.tile([P, FD], f16)
            d0_16 = pp.tile([P, FD], f16)      # raw delta0 (step-0 operand)
            halfpi_b = pp.tile([P, 1], f32)
            nc.gpsimd.memset(halfpi_b[:], float(HALF_PI))
            # Pin the ACT table set (silu_and_others holds Sin/Tanh/Identity)
            nc.scalar.activation(halfpi_b[:], halfpi_b[:], ACT.Silu)
            nc.gpsimd.memset(halfpi_b[:], float(HALF_PI))

            # ---------- PSUM ----------
            XY_q = qq.tile([P, 2 * FD], f32)     # [X | Y] tanh args (scratch)
            S1_pad = qq.tile([P, 2 * FD], f32)   # S1 alone in its bank
            S1_q = S1_pad[:, 0:FD]
            R_pad = qq.tile([P, 2 * FD], f32)    # rhat alone in its bank
            R_q = R_pad[:, 0:FD]
            PB_q = qq.tile([P, 2 * FD], f32)     # [psi | beta], never reset
            psi_q = PB_q[:, 0:FD]
            beta_q = PB_q[:, FD:2 * FD]
            xy_q = qq.tile([P, 2 * FD], f32)     # [x | y]

            def W(i):
                return wsb[:, bass.ts(i, P)]

            def mm(out_ap, didx, rhs_ap, start, stop):
                nc.tensor.matmul(out_ap, W(didx), rhs_ap, start=start,
                                 stop=stop, skip_group_check=True)

            # ---------------- load + unpack ----------------
            nc.sync.dma_start(big_in[:], st_d[:].rearrange(
                "(p q) v -> p (q v)", p=P))
            nc.sync.dma_start(big_ac[:], ac_d[:].rearrange(
                "(p q) v -> p (q v)", p=P))
            nc.sync.dma_start(wsb[:].rearrange("p (d m) -> p d m", m=P),
                              wd_d[:].rearrange("d k m -> k d m"))

            sv = big_in[:].rearrange("p (q v) -> p q v", v=9)
            av = big_ac[:].rearrange("p (q v) -> p q v", v=2)

            # clips of action
            aref_c = sp.tile([P, FD], f32, tag="aref")
            nc.vector.tensor_scalar(aref_c[:], av[:, :, 0], float(MIN_ACC),
                                    float(MAX_ACC), alu.max, alu.min)
            nc.vector.tensor_scalar(dref_c[:], av[:, :, 1], float(-MS),
                                    float(MS), alu.max, alu.min)
            nc.vector.tensor_copy(dref16[:], dref_c[:])

            # v0, hA0 = h*a0, CA = (h*(h/tau))*aref
            nc.vector.tensor_copy(v_t[0][:], sv[:, :, 3])
            nc.vector.tensor_scalar(hA_t[0][:], sv[:, :, 4], h_base, None,
                                    alu.mult)
            ca_s = float(_f32(h_base) * _f32(h_base / TAU_A))
            nc.vector.tensor_scalar(CA32[:], aref_c[:], ca_s, None, alu.mult)



            # delta: d1 = clip(QD1*d0 + (1-QD1)*dref); D = d1 - dref
            d1_t = sp.tile([P, FD], f32, tag="d1")
            cd1 = float(_f32(float(hs32[0]) / TAU_D))
            nc.vector._custom_dve(DCLIP, out=d1_t[:], in0=sv[:, :, 5],
                                  in1=dref_c[:], s0=QD[0], s1=float(-MS),
                                  imm2=cd1)
            nc.vector.tensor_tensor(D32[:], d1_t[:], dref_c[:], alu.subtract)
            nc.vector.tensor_copy(D16[:], D32[:])
            nc.vector.tensor_copy(d0_16[:], sv[:, :, 5])

            # phi0 = wrap(psi0 + beta0) -> fp16
            pb0 = sp.tile([P, FD], f32, tag="pb0")
            nc.vector.tensor_tensor(pb0[:], sv[:, :, 2], sv[:, :, 6], alu.add)
            nc.vector._custom_dve(WRAPRN, out=phi_t[0][:], in0=pb0[:],
                                  s0=float(INV_2PI), s1=float(MAGIC),
                                  imm2=float(TWO_PI))

            # PSUM seeds via fp16 hi/lo matmul pairs (exact to ~2^-22)
            xy0 = sp.tile([P, 2 * FD], f32, tag="xy0")
            nc.vector.tensor_copy(xy0[:, 0:FD], sv[:, :, 0])
            nc.vector.tensor_copy(xy0[:, FD:2 * FD], sv[:, :, 1])
            pbs0 = sp.tile([P, 2 * FD], f32, tag="pbs0")
            nc.vector.tensor_copy(pbs0[:, 0:FD], sv[:, :, 2])
            nc.vector.tensor_copy(pbs0[:, FD:2 * FD], sv[:, :, 6])

            def seed(q_ap, src_ap, w, tagp):
                hi = sp.tile([P, w], f16, tag=tagp + "hi")
                lo = sp.tile([P, w], f16, tag=tagp + "lo")
                nc.vector.tensor_copy(hi[:], src_ap)
                nc.vector.tensor_tensor(lo[:], src_ap, hi[:], alu.subtract)
                mm(q_ap, D_ONE, hi[:], start=True, stop=False)
                mm(q_ap, D_ONE, lo[:], start=False, stop=False)

            seed(xy_q[:], xy0[:], 2 * FD, "sxy")
            seed(R_q, sv[:, :, 7], FD, "sr")
            seed(PB_q[:], pbs0[:], 2 * FD, "spb")

            # ---------------- main loop ----------------
            xy_pending = []
            for k in range(n_steps):
                h = float(hs32[k])
                hd = hdi[h]
                p = k % 2
                pn = (p + 1) % 2
                fp = k % 3
                fpn = (k + 1) % 3
                v_p = v_t[k % 8]
                v_pn = v_t[(k + 1) % 8]
                last = (k + 1 == n_steps)

                # hr16 = h*rhat_k (ACT identity with scale; before r-mms)
                hr16 = sp.tile([P, FD], f16, tag="hr16")
                nc.scalar.activation(hr16[:], R_q, ACT.Identity, scale=h)

                # trig: sin of phi_k (leaf path: feeds only vcvs/xy)
                trig = sp.tile([P, 2 * FD], f16, tag="trig")  # [cos | sin]
                absphi = sp.tile([P, FD], f16, tag="absphi")
                nc.vector.tensor_scalar(absphi[:].bitcast(u16),
                                        phi_t[fp][:].bitcast(u16),
                                        0x7FFF, None, alu.bitwise_and)
                nc.scalar.activation(trig[:, FD:2 * FD], phi_t[fp][:], ACT.Sin)

                # beta16 mirror of beta_k (before this step's beta mms)
                beta16 = sp.tile([P, FD], f16, tag="beta16")
                nc.scalar.copy(beta16[:], beta_q)

                # cos fills the ACT gap before tanh
                nc.scalar.activation(trig[:, 0:FD], absphi[:], ACT.Sin,
                                     bias=halfpi_b[:], scale=-1.0)

                # early off-cycle mms: psi += h*r, beta -= h*r
                mm(psi_q, D_ONE, hr16[:], start=False, stop=last)
                mm(beta_q, D_NEG1, hr16[:], start=False, stop=False)

                # vcvs = v*[cos|sin] on gpsimd (lagging leaf; its xy-mm is
                # emitted 2 steps later so the PE queue never waits on it)
                vcvs = sp.tile([P, 2 * FD], f16, tag="vcvs")
                nc.gpsimd.tensor_tensor(
                    vcvs[:].rearrange("p (a b) -> p a b", a=2),
                    trig[:].rearrange("p (a b) -> p a b", a=2),
                    v_p[:].unsqueeze(1).broadcast_to([P, 2, FD]),
                    alu.mult)
                xy_pending.append((vcvs, hd["XY"]))

                # reciprocal of v_eff (seedless)
                ve = sp.tile([P, FD], f32, tag="ve")
                inv = sp.tile([P, FD], f32, tag="inv")
                nc.vector.tensor_scalar(ve[:], v_p[:], float(VMIN), None,
                                        alu.max)
                nc.vector.reciprocal_approx_fast(out=inv[:], in_=ve[:])

                # rv16 = (h*r)*inv  (X/Y weights absorb LF/h etc.)
                rv16 = sp.tile([P, FD], f16, tag="rv16")
                nc.vector.tensor_tensor(rv16[:], hr16[:], inv[:], alu.mult)

                # ---- X,Y tanh args (PSUM scratch) ----
                # NOTE: start=True zeroes the WHOLE bank -> exactly one
                # start=True mm per bank per step, ordered first.
                # order: delta/dref/rv first, beta LAST (shortest cycle)
                if k == 0:
                    mm(XY_q[:, 0:FD], D_NCF, d0_16[:], start=True, stop=False)
                else:
                    mm(XY_q[:, 0:FD], D_NCF, dref16[:], start=True,
                       stop=False)
                    mm(XY_q[:, 0:FD], D_DEC[k - 1], D16[:], start=False,
                       stop=False)
                mm(XY_q[:, FD:2 * FD], hd["RVR"], rv16[:], start=False,
                   stop=False)
                mm(XY_q[:, 0:FD], hd["RVF"], rv16[:], start=False, stop=False)
                mm(XY_q[:, FD:2 * FD], D_CR, beta16[:], start=False,
                   stop=True)
                mm(XY_q[:, 0:FD], D_CF, beta16[:], start=False, stop=True)

                # one contiguous 2FD tanh -> [Tf | Tr] fp16
                TfTr = sp.tile([P, 2 * FD], f16, tag="TfTr")
                nc.scalar.activation(TfTr[:], XY_q[:], ACT.Tanh)
                Tf = TfTr[:, 0:FD]
                Tr = TfTr[:, FD:2 * FD]

                # S1 (scratch) and rhat (accumulator)
                mm(S1_q, hd["C1"], Tf, start=True, stop=False)
                mm(S1_q, hd["C2"], Tr, start=False, stop=True)
                mm(R_q, hd["K1"], Tf, start=False, stop=False)
                mm(R_q, hd["K2"], Tr, start=False, stop=True)

                # w16 = S1 * inv
                w16 = sp.tile([P, FD], f16, tag="w16")
                nc.vector.tensor_tensor(w16[:], S1_q, inv[:], alu.mult)

                if DBG and k == 0:
                    dbg_sb = pp.tile([P, FD * 8], f32, name="dbg_sb")
                    dv = dbg_sb[:].rearrange("p (q v) -> p q v", v=8)
                    nc.vector.tensor_copy(dv[:, :, 0], XY_q[:, 0:FD])
                    nc.vector.tensor_copy(dv[:, :, 1], XY_q[:, FD:2 * FD])
                    nc.vector.tensor_copy(dv[:, :, 2], TfTr[:, 0:FD])
                    nc.vector.tensor_copy(dv[:, :, 3], TfTr[:, FD:2 * FD])
                    nc.vector.tensor_copy(dv[:, :, 4], rv16[:])
                    nc.vector.tensor_copy(dv[:, :, 5], hr16[:])
                    nc.vector.tensor_copy(dv[:, :, 6], inv[:])
                    nc.vector.tensor_copy(dv[:, :, 7], beta16[:])
                    nc.sync.dma_start(
                        dbg_d[:].rearrange("(p q) v -> p (q v)", p=P),
                        dbg_sb[:])

                # beta += w (cycle-critical mm)
                mm(beta_q, D_ONE, w16[:], start=False, stop=last)
                # deferred xy-mm from 2 steps ago
                if len(xy_pending) > 2:
                    vc_o, w_o = xy_pending.pop(0)
                    mm(xy_q[:], w_o, vc_o[:], start=False, stop=False)

                # phi' = phi + w (wrap every WRAP_EVERY steps)
                if k % WRAP_EVERY == WRAP_EVERY - 1:
                    nc.vector._custom_dve(PHISTEP, out=phi_t[fpn][:],
                                          in0=phi_t[fp][:], in1=w16[:],
                                          s0=float(PI_F),
                                          imm2=float(TWO_PI))
                else:
                    nc.vector.tensor_tensor(phi_t[fpn][:], phi_t[fp][:],
                                            w16[:], alu.add)

                # v' = relu(v + hA*(h/h_base));  hA' = QA*hA + CA
                nc.vector._custom_dve(RELUADD, out=v_pn[:],
                                      in0=v_p[:], in1=hA_t[p][:],
                                      s0=float(_f32(h) / _f32(h_base)))
                nc.vector.scalar_tensor_tensor(hA_t[pn][:], hA_t[p][:],
                                               QA[k], CA32[:], alu.mult,
                                               alu.add)

            # flush remaining deferred xy-mms
            for i, (vc_o, w_o) in enumerate(xy_pending):
                mm(xy_q[:], w_o, vc_o[:], start=False,
                   stop=(i + 1 == len(xy_pending)))

            # ---------------- finalize ----------------
            pl = n_steps % 2
            ov = big_out[:].rearrange("p (q v) -> p q v", v=9)
            nc.vector.tensor_copy(ov[:, :, 0], xy_q[:, 0:FD])
            nc.vector.tensor_copy(ov[:, :, 1], xy_q[:, FD:2 * FD])
            nc.scalar.copy(ov[:, :, 2], psi_q)
            inv_hb = float(_f32(1.0) / _f32(h_base))
            nc.vector.tensor_copy(ov[:, :, 3], v_t[n_steps % 8][:])
            nc.vector.tensor_scalar(ov[:, :, 4], hA_t[pl][:], inv_hb, None,
                                    alu.mult)
            # delta_N = dref + ppi[N-1]*QD[N-1]... = dref + (prod all QD)*D
            pN = ppi[-1] * QD[-1] if n_steps >= 2 else QD[0] * 1.0
            # careful: delta after step k uses prod_{j=1..k} ... the OUTPUT
            # delta is delta_{n_steps} = dref + (prod_{j=1}^{n_steps-1} QD_j)
            # * QD_? ... d_{k+1} = dref + QD_{k}*(d_k - dref) for k>=1, so
            # d_N = dref + (prod_{j=1}^{N-1} QD_j) * D  with D = d_1 - dref.
            pN = 1.0
            for j in range(1, n_steps):
                pN *= QD[j]
            nc.vector.scalar_tensor_tensor(ov[:, :, 5], D32[:], float(pN),
                                           dref_c[:], alu.mult, alu.add)
            nc.scalar.copy(ov[:, :, 6], beta_q)
            nc.scalar.copy(ov[:, :, 7], R_q)
            nc.vector.tensor_copy(ov[:, :, 8], dref_c[:])
            nc.sync.dma_start(out_d[:].rearrange("(p q) v -> p (q v)", p=P),
                              big_out[:])

    nc.compile()
    return nc, wdiag_host


_BUILD_CACHE = {}


def _get_built(dt_total, n_veh=B_CORE):
    hs = tuple(_step_hs(float(dt_total)))
    key = (hs, n_veh)
    if key not in _BUILD_CACHE:
        _BUILD_CACHE[key] = build_kernel(list(hs), n_veh)
    return _BUILD_CACHE[key]


def kernel(state, action, dt):
    state = np.ascontiguousarray(np.asarray(state, dtype=np.float32))
    action = np.ascontiguousarray(np.asarray(action, dtype=np.float32))
    assert state.shape == (B_TOTAL, 9) and action.shape == (B_TOTAL, 2)

    nc, wdiag = _get_built(float(dt))

    from concourse.bass_utils import run_bass_kernel_spmd

    st_sh = np.split(state, N_CORES, axis=0)
    ac_sh = np.split(action, N_CORES, axis=0)
    in_maps = [
        {"state": np.ascontiguousarray(st_sh[i]),
         "action": np.ascontiguousarray(ac_sh[i]),
         "wdiag": wdiag}
        for i in range(N_CORES)
    ]
    res = run_bass_kernel_spmd(nc, in_maps, core_ids=list(range(N_CORES)))
    out = np.concatenate([r["out"] for r in res.results], axis=0)
    return out.astype(np.float32)


if __name__ == "__main__":
    rng = np.random.default_rng(0)
    s = rng.standard_normal((B_TOTAL, 9), dtype=np.float32)
    a = rng.standard_normal((B_TOTAL, 2), dtype=np.float32)
    o = kernel(s, a, 1)
    print("out", o.shape, o.dtype, np.isfinite(o).all())
